# revision 21
# baseline (speedup 1.0000x reference)
# CBAM (channel + spatial attention) Trainium2 kernel.
#
# Full inputs:  x [32, 512, 56, 56] f32, w1 [32, 512], w2 [512, 32],
#               w_conv [1, 2, 7, 7].
# Sharding: data-parallel over batch, 4 samples per core on 8 cores,
# params replicated. One SPMD Bass/Tile program; inputs sharded /
# outputs gathered here.
import numpy as np

import concourse.bacc as bacc
import concourse.bass as bass
import concourse.bass_isa as bass_isa
import concourse.mybir as mybir
import concourse.tile as tile
from concourse.bass_utils import run_bass_kernel_spmd

F32 = mybir.dt.float32
AF = mybir.ActivationFunctionType
ALU = mybir.AluOpType
AX = mybir.AxisListType

N_CORES = 8
B, C, H, W = 32, 512, 56, 56
B_LOC = B // N_CORES          # 4 samples per core
P = 128                       # SBUF partitions
NT = C // P                   # 4 channel tiles per sample
HW = H * W                    # 3136
PADW = W + 6                  # 62 (7x7 conv, pad 3)
PADHW = PADW * PADW           # 3844
NCH = 7                       # spatial chunks
CHUNK = HW // NCH             # 448 pixels = 8 rows of 56
ROWS_PER_CHUNK = CHUNK // W   # 8
R_HID = 32                    # C // 16


def _body(tc, x_d, w1_d, w2_d, wc_d, out_d):
    nc = tc.nc

    with (
        tc.tile_pool(name="singles", bufs=1) as singles,
        tc.tile_pool(name="xpool", bufs=2) as xpool,
        tc.tile_pool(name="mpool", bufs=1) as mpool,
        tc.tile_pool(name="spool", bufs=1) as spool,
        tc.tile_pool(name="small", bufs=2) as small,
        tc.tile_pool(name="ps_mlp", bufs=2, space="PSUM") as ps_mlp,
        tc.tile_pool(name="ps_ss", bufs=2, space="PSUM") as ps_ss,
        tc.tile_pool(name="ps_cv", bufs=2, space="PSUM") as ps_cv,
    ):
        # ---- per-core constants ----
        # w1T [512, 32] as 4 chunks [128, 32]  (rhs of mlp mm1)
        w1t = singles.tile([P, NT, R_HID], F32)
        w1_r = w1_d.rearrange("r (c p) -> p c r", p=P)
        for i in range(NT):
            nc.sync.dma_start(out=w1t[:, i, :], in_=w1_r[:, i, :])
        # w2T [32, 512] as 4 chunks [32, 128]  (lhsT of mlp mm2)
        w2t = singles.tile([R_HID, NT, P], F32)
        w2_r = w2_d.rearrange("(c p) r -> r c p", p=P)
        for i in range(NT):
            nc.sync.dma_start(out=w2t[:, i, :], in_=w2_r[:, i, :])
        # conv weights broadcast: [98, 128], every column = w_conv flattened
        # in (dx, ci, dy) order (so each dx is a contiguous 14-row block)
        wcb = singles.tile([2 * 49, P], F32)
        for dx in range(7):
            nc.sync.dma_start(
                out=wcb[14 * dx : 14 * dx + 14, 0:1],
                in_=bass.AP(
                    tensor=wc_d.tensor,
                    offset=wc_d.offset + dx,
                    ap=[[49, 2], [7, 7], [1, 1]],
                ),
            )
        k = 1
        while k < P:
            nc.vector.tensor_copy(out=wcb[:, k : 2 * k], in_=wcb[:, 0:k])
            k *= 2
        EDGE_DX = (0, 1, 2, 4, 5, 6)
        # y-only padded spatial maps: [2, 4 + 62*56 + 4]; (y,x) contiguous.
        # data row y at offset 172 + 56*y; 3 zero rows + 4 zero elems each side
        padded = singles.tile([2, 8 + 62 * W], F32)
        nc.vector.memset(padded, 0.0)
        # contiguous zeros for the im2col edge-zeroing DMAs (max 14*H*3 elems)
        zeros = singles.tile([1, 14 * H * 3], F32)
        nc.vector.memset(zeros, 0.0)

        for s in range(B_LOC):
            x_dram = x_d[s].rearrange("(c p) h w -> p c (h w)", p=P)
            o_dram = out_d[s].rearrange("(c p) h w -> p c (h w)", p=P)

            x_sb = xpool.tile([P, NT, HW], F32, tag="x")
            for i in range(NT):
                nc.sync.dma_start(out=x_sb[:, i, :], in_=x_dram[:, i, :])

            # ---- channel stats: sum (ACT, fused w/ self-copy) + max (DVE)
            st = small.tile([P, NT, 2], F32, tag="st")
            for i in range(NT):
                nc.scalar.activation(
                    out=x_sb[:, i, :],
                    in_=x_sb[:, i, :],
                    func=AF.Copy,
                    accum_out=st[:, i, 0:1],
                )
                nc.vector.reduce_max(
                    out=st[:, i, 1:2], in_=x_sb[:, i, :], axis=AX.X
                )
            # sum -> mean
            nc.vector.tensor_scalar_mul(
                out=st[:, :, 0], in0=st[:, :, 0], scalar1=1.0 / HW
            )

            # ---- tiny MLP:  h = relu(v @ w1T)  [2, 32] ----
            ph = ps_mlp.tile([2, R_HID], F32, tag="ph")
            for i in range(NT):
                nc.tensor.matmul(
                    ph,
                    lhsT=st[:, i, :],
                    rhs=w1t[:, i, :],
                    start=(i == 0),
                    stop=(i == NT - 1),
                )
            hpad = small.tile([R_HID, R_HID], F32, tag="hpad")
            nc.vector.memset(hpad, 0.0)
            nc.scalar.activation(out=hpad[0:2, :], in_=ph, func=AF.Relu)
            ht = small.tile([R_HID, R_HID], F32, tag="ht")
            nc.vector.transpose(out=ht, in_=hpad)
            # mm2: [128, 2] per chunk = (w2 @ h) ; add paths; sigmoid
            po = ps_mlp.tile([P, NT, 2], F32, tag="po")
            for i in range(NT):
                nc.tensor.matmul(
                    po[:, i, :],
                    lhsT=w2t[:, i, :],
                    rhs=ht[:, 0:2],
                    start=True,
                    stop=True,
                )
            co = small.tile([P, NT, 2], F32, tag="co")
            nc.scalar.copy(out=co, in_=po)
            ca = small.tile([P, NT], F32, tag="ca")
            nc.vector.tensor_add(out=ca, in0=co[:, :, 0], in1=co[:, :, 1])
            nc.scalar.activation(out=ca, in_=ca, func=AF.Sigmoid)

            # ---- spatial sum of scaled x on PE (fused channel-scale) ----
            for j in range(NCH):
                pss = ps_ss.tile([1, CHUNK], F32, tag="pss")
                for i in range(NT):
                    nc.tensor.matmul(
                        pss,
                        lhsT=ca[:, i : i + 1],
                        rhs=x_sb[:, i, j * CHUNK : (j + 1) * CHUNK],
                        start=(i == 0),
                        stop=(i == NT - 1),
                    )
                # mean over C into padded row 0 (contiguous interior)
                nc.scalar.mul(
                    out=padded[0:1, 172 + j * CHUNK : 172 + (j + 1) * CHUNK],
                    in_=pss,
                    mul=1.0 / C,
                )

            # ---- scale x in place (split ACT / DVE) ----
            nc.scalar.mul(out=x_sb[:, 0, :], in_=x_sb[:, 0, :], mul=ca[:, 0:1])
            nc.scalar.mul(out=x_sb[:, 1, :], in_=x_sb[:, 1, :], mul=ca[:, 1:2])
            nc.scalar.mul(out=x_sb[:, 2, :], in_=x_sb[:, 2, :], mul=ca[:, 2:3])
            nc.vector.tensor_scalar_mul(
                out=x_sb[:, 3, :], in0=x_sb[:, 3, :], scalar1=ca[:, 3:4]
            )

            # ---- spatial max over C: tree folds + partition all-reduce ----
            m1a = mpool.tile([P, HW], F32, tag="m1a")
            m1b = mpool.tile([P, HW], F32, tag="m1b")
            nc.vector.tensor_tensor(
                out=m1a, in0=x_sb[:, 0, :], in1=x_sb[:, 1, :], op=ALU.max
            )
            nc.vector.tensor_tensor(
                out=m1b, in0=x_sb[:, 2, :], in1=x_sb[:, 3, :], op=ALU.max
            )
            nc.vector.tensor_tensor(out=m1a, in0=m1a, in1=m1b, op=ALU.max)
            nc.gpsimd.partition_all_reduce(
                out_ap=m1b, in_ap=m1a, channels=P, reduce_op=bass_isa.ReduceOp.max
            )
            # max map into padded row 1 (contiguous interior); DMA because
            # compute engines cannot write with base partition 1
            nc.sync.dma_start(
                out=padded[1:2, 172 : 172 + HW], in_=m1b[0:1, :]
            )

            # ---- 7x7 conv as K=98 matmul over DMA-gathered im2col ----
            # hop 1: row-shifted maps RS[(ci,dy), e] = padded[ci, 1+56*dy+e]
            rs = spool.tile([14, HW + 6], F32, tag="rs")
            hop1_base = padded[:, 1 : 1 + HW + 6]
            nc.sync.dma_start(
                out=rs,
                in_=bass.AP(
                    tensor=hop1_base.tensor,
                    offset=hop1_base.offset,
                    ap=[hop1_base.ap[0], [W, 7], hop1_base.ap[1]],
                ),
            )
            # hop 2: im2col[(dx,ci,dy), n] = RS[(ci,dy), dx + n], one DMA per dx
            i2c = spool.tile([2 * 49, HW], F32, tag="i2c")
            for dx in range(7):
                nc.sync.dma_start(
                    out=i2c[14 * dx : 14 * dx + 14, :],
                    in_=rs[:, dx : dx + HW],
                )
            # zero the x-wraparound entries so the conv matmul is exact:
            # row block dx, columns x<3-dx (dx<3) or x>=59-dx (dx>3)
            for dx in EDGE_DX:
                x0, nx = (0, 3 - dx) if dx < 3 else (59 - dx, dx - 3)
                dstv = i2c[14 * dx : 14 * dx + 14, :].rearrange(
                    "k (y x) -> k y x", x=W
                )[:, :, x0 : x0 + nx]
                nc.sync.dma_start(
                    out=dstv,
                    in_=bass.AP(
                        tensor=zeros.tensor,
                        offset=zeros.offset,
                        ap=[[zeros.ap[0][0], 1], [nx, 14 * H], [1, nx]],
                    ),
                )
            # conv matmuls; lhsT = w broadcast to 128 cols, so the matmul
            # itself replicates sp_att to all partitions.
            sp = spool.tile([P, HW], F32, tag="sp")
            for j in range(NCH):
                pcv = ps_cv.tile([P, CHUNK], F32, tag="pcv")
                nc.tensor.matmul(
                    pcv,
                    lhsT=wcb,
                    rhs=i2c[:, j * CHUNK : (j + 1) * CHUNK],
                    start=True,
                    stop=True,
                )
                nc.scalar.activation(
                    out=sp[:, j * CHUNK : (j + 1) * CHUNK],
                    in_=pcv,
                    func=AF.Sigmoid,
                )

            # ---- final multiply + store ----
            for i in range(NT):
                nc.vector.tensor_mul(
                    out=x_sb[:, i, :], in0=x_sb[:, i, :], in1=sp
                )
                nc.sync.dma_start(out=o_dram[:, i, :], in_=x_sb[:, i, :])


_NC_CACHE = {}


def _get_nc():
    if "nc" not in _NC_CACHE:
        nc = bacc.Bacc(
            "TRN2",
            target_bir_lowering=False,
            debug=False,
            enable_asserts=False,
            num_devices=N_CORES,
        )
        x_d = nc.dram_tensor("x", [B_LOC, C, H, W], F32, kind="ExternalInput").ap()
        w1_d = nc.dram_tensor("w1", [C // 16, C], F32, kind="ExternalInput").ap()
        w2_d = nc.dram_tensor("w2", [C, C // 16], F32, kind="ExternalInput").ap()
        wc_d = nc.dram_tensor("w_conv", [1, 2, 7, 7], F32, kind="ExternalInput").ap()
        out_d = nc.dram_tensor("out", [B_LOC, C, H, W], F32, kind="ExternalOutput").ap()
        with tile.TileContext(nc) as tc:
            _body(tc, x_d, w1_d, w2_d, wc_d, out_d)
        nc.compile()
        _NC_CACHE["nc"] = nc
    return _NC_CACHE["nc"]


def kernel(x, w1, w2, w_conv):
    nc = _get_nc()
    x = np.ascontiguousarray(x, dtype=np.float32)
    in_maps = [
        {
            "x": np.ascontiguousarray(x[i * B_LOC : (i + 1) * B_LOC]),
            "w1": np.ascontiguousarray(w1, dtype=np.float32),
            "w2": np.ascontiguousarray(w2, dtype=np.float32),
            "w_conv": np.ascontiguousarray(w_conv, dtype=np.float32),
        }
        for i in range(N_CORES)
    ]
    res = run_bass_kernel_spmd(nc, in_maps, core_ids=list(range(N_CORES)))
    return np.concatenate([r["out"] for r in res.results], axis=0)
